# revision 65
# baseline (speedup 1.0000x reference)
"""Trainium2 Bass kernel for PVT-style spatial-reduction attention.

Shapes (hardcoded): x [2, 4096, 256], HEAD=8, dh=32, SR=2, R=8, H=W=64.
Sharding: core c = (batch b = c//4, query block j = c%4). Each core computes
q/attention/proj for its 1024 query rows and redundantly computes the small
conv+LN+KV path for its batch (no collectives). Per-core x is pre-rotated on
host so each core's own query block is rows 0:1024.

Structure (driven by the TimelineSim cost model; 147us baseline -> 101us):
- The Activation engine's exp stream (64 x [128,1024] tiles ~= 66us) is the
  hard floor; everything else is arranged to keep it 100% fed (total exp
  stalls ~1.5us).
- Score matmuls run in fp8e4m3 DoubleRow (half cost, PE gains the slack
  that previously starved the exp stream). q/k are scaled x4 host-side to
  stay out of fp8 subnormals; the exp scale divides the 16x back out with
  the per-token LayerNorm rstd. Weight output columns are host-permuted so
  the q/k PSUM rows come out (dlo, ktile, head)-ordered and one partition-
  folding SBUF->SBUF DMA produces the [16, 2, ...] DoubleRow layout.
- Both LoRA adapters are folded into their base weights host-side
  (W + B@A, exact); LayerNorm gamma/mean fold into the kv weights, v-side
  constants into the proj bias (rank-1 ones-row matmul).
- Weights packed into a few blob DMAs (HWDGE serializes ~1.3us per DMA),
  ordered so conv's inputs land first; PE warmup matmuls during the DMA
  phase hold the p-state at full clock for conv.
- Strip-0 attention starts as soon as strip-0 K + Q exist; strip-1
  conv/LN/KV is interleaved into the PE queue between early score tiles.
- LN-stat repack and softmax-denominator broadcast use PE transposes /
  ones-matmuls instead of DRAM round-trips; normalized outputs are written
  by the DVE directly into the outT layout (partition rebase, no DMA).
- Attention is software-pipelined: pv trails scores by 4 steps (2 for the
  last group) so group-boundary denominator chains hide under the exp
  cushion; the output projection/staging splits across PE/Act/DVE with
  paired output DMAs.

PSUM budget (8 banks): scores 2x[128,1024] (4) + pv 2x[128,512] (2) +
lane A (1) + lane B (1); lanes A/B carry conv/stats/kv/vT in setup and
recb/proj afterwards.
"""
import sys

if "/opt/trn_rl_repo" not in sys.path:
    sys.path.insert(0, "/opt/trn_rl_repo")

import numpy as np
import ml_dtypes

BF16NP = ml_dtypes.bfloat16

HEAD, DH, C, N, B, M, R = 8, 32, 256, 4096, 2, 1024, 8
NB = N // 4          # query rows per core
SCALE = DH ** -0.5
NCORES = 8
MAGIC = 0x5F3759DF

_CACHE = {}


def _build_program():
    import concourse.bass as bass
    import concourse.tile as tile
    from concourse.bacc import Bacc
    from concourse import mybir, masks

    F32 = mybir.dt.float32
    F32R = mybir.dt.float32r
    FP8 = mybir.dt.float8e4
    DR = mybir.MatmulPerfMode.DoubleRow
    BF16 = mybir.dt.bfloat16
    I32 = mybir.dt.int32
    AF = mybir.ActivationFunctionType
    ALU = mybir.AluOpType

    nc = Bacc()
    P = 128
    S = 2          # kv strips
    ST = 512       # kv tokens per strip

    # ---- DRAM parameters (host-prepped packed layouts) ----
    xT_d = nc.declare_dram_parameter("xT", [P, 2, N], BF16, isOutput=False)
    wA_d = nc.declare_dram_parameter("wA", [P, 2048], BF16, isOutput=False)
    wB1_d = nc.declare_dram_parameter("wB1", [P, 512], BF16, isOutput=False)
    wB2_d = nc.declare_dram_parameter("wB2", [P, 2432], BF16, isOutput=False)
    wC_d = nc.declare_dram_parameter("wC", [P, 4], F32, isOutput=False)
    out_d = nc.declare_dram_parameter("out", [NB, C], F32, isOutput=True)

    with tile.TileContext(nc) as tc:
        with tc.tile_pool(name="wgt", bufs=1) as WGT, \
             tc.tile_pool(name="acts", bufs=1) as ACTS, \
             tc.tile_pool(name="strips", bufs=2) as STR, \
             tc.tile_pool(name="tmp", bufs=3) as TMP, \
             tc.tile_pool(name="atn", bufs=2) as ATN, \
             tc.tile_pool(name="pt", bufs=32) as PT, \
             tc.tile_pool(name="fin", bufs=2) as FIN, \
             tc.tile_pool(name="big", bufs=2, space="PSUM") as PSB, \
             tc.tile_pool(name="pv", bufs=2, space="PSUM") as PSV, \
             tc.tile_pool(name="la", bufs=1, space="PSUM") as PSC, \
             tc.tile_pool(name="lb", bufs=1, space="PSUM") as PSK:

            # ---------- warmups + loads ----------
            # dummy exp pulls the Exp ACT_TABLE_LOAD off the critical path
            warm = WGT.tile([1, 2], F32, tag="warm")
            nc.gpsimd.memset(warm[:], 0.0)
            warmo = WGT.tile([1, 2], F32, tag="warmo")
            nc.scalar.activation(out=warmo[:], in_=warm[:], func=AF.Exp)
            # matmul fodder to ramp the PE p-state during the DMA phase
            wz = WGT.tile([P, 256], BF16, tag="wz")
            nc.gpsimd.memset(wz[:], 0.0)

            wA = WGT.tile([P, 2048], BF16, tag="wA")
            nc.sync.dma_start(out=wA[:], in_=wA_d[:])
            xs0 = ACTS.tile([P, 2, 2048], BF16, tag="xT0")
            nc.sync.dma_start(out=xs0[:, 0, :], in_=xT_d[:, 0, 0:2048])
            nc.sync.dma_start(out=xs0[:, 1, :], in_=xT_d[:, 1, 0:2048])
            wB1 = WGT.tile([P, 512], BF16, tag="wB1")
            nc.sync.dma_start(out=wB1[:], in_=wB1_d[:])
            wC = WGT.tile([P, 4], F32, tag="wC")
            nc.sync.dma_start(out=wC[:], in_=wC_d[:])
            wB2 = WGT.tile([P, 2432], BF16, tag="wB2")
            nc.sync.dma_start(out=wB2[:], in_=wB2_d[:])
            xs1 = ACTS.tile([P, 2, 2048], BF16, tag="xT1")
            nc.sync.dma_start(out=xs1[:], in_=xT_d[:, :, 2048:4096])
            xTs = [xs0, xs1]

            srwT = wA[:, 0:2048].rearrange("p (c f k) -> p c f k", c=2, f=4)
            qwT = wB1[:, 0:512].rearrange("p (c k) -> p c k", c=2)
            kvwT = wB2[:, 0:1024].rearrange("p (c k) -> p c k", c=2)
            pwT = wB2[:, 1024:1536].rearrange("p (c k) -> p c k", c=2)
            wg1t = wB2[0:1, 1536:2048].rearrange("p (g k) -> p g k", g=4)
            onesb = wB2[0:1, 2048:2176]
            pbb = wB2[0:1, 2176:2432]
            ones32 = onesb[0:1, 0:32]
            qb = wC[:, 0:2]
            srb = wC[:, 2:4]

            ones1 = WGT.tile([P, 1], BF16, tag="ones1")
            nc.gpsimd.memset(ones1[:], 1.0 / C)
            ident = WGT.tile([P, P], BF16, tag="ident")
            masks.make_identity(nc, ident[:])

            # keep the PE continuously busy through the DMA phase so the
            # p-state ramp (full speed after 3us busy) is done before conv
            for _ in range(16):
                wp = PSC.tile([1, 256], F32, tag="la", name="wp")
                nc.tensor.matmul(wp[:], wz[:, 0:1], wz[:], start=True, stop=True)

            # persistent activations; q/k live in the fp8 DoubleRow layout
            # [16 (dh lo), chunk, 2 (dh k-tile), 4 (head in chunk), tokens].
            # The projection weight columns are host-permuted so the PSUM
            # rows come out (d, kt, hl)-ordered and a single SBUF->SBUF DMA
            # folds 128 partitions into 16 x 8.
            q8 = ACTS.tile([16, 2, 2, 4, NB], FP8, tag="q8")
            outT = ACTS.tile([P, 2, NB], BF16, tag="outT")

            xsb, nm2s, kts, vtmp, vsb, ans, ascl = {}, {}, {}, {}, {}, {}, {}
            lane = [0]

            def lane_tile(shape, dt):
                pool = PSC if lane[0] % 2 == 0 else PSK
                tg = "la" if pool is PSC else "lb"
                lane[0] += 1
                return pool.tile(shape, dt, tag=tg, name=f"ln{lane[0]}")

            # ---------- per-strip setup pieces ----------
            def conv(s, ocs=(0, 1), cc_major=False):
                # 2x2 stride-2 conv as 8 accumulated matmuls per out-chunk;
                # +srb fold straight to bf16
                xs_t = xTs[s]
                if s not in xsb:
                    xsb[s] = STR.tile([P, 2, ST], BF16, tag="xsb", name=f"xsb{s}")
                xsb_s = xsb[s]
                cps = {oc: lane_tile([P, ST], F32) for oc in ocs}
                order = ([(cc, oc, di, dj) for cc in range(2) for oc in ocs
                          for di in range(2) for dj in range(2)]
                         if cc_major else
                         [(cc, oc, di, dj) for oc in ocs for cc in range(2)
                          for di in range(2) for dj in range(2)])
                seen = {}
                for cc, oc, di, dj in order:
                    xv = xs_t[:, cc, :].rearrange(
                        "p (i a j b) -> p i a j b", i=16, a=2, j=32, b=2)
                    k = seen.get(oc, 0)
                    seen[oc] = k + 1
                    nc.tensor.matmul(
                        cps[oc][:], srwT[:, cc, di * 2 + dj,
                                         oc * P:(oc + 1) * P],
                        xv[:, :, di, :, dj],
                        start=(k == 0), stop=(k == 7))
                for oc in ocs:
                    nc.vector.tensor_scalar_add(
                        out=xsb_s[:, oc, :], in0=cps[oc][:],
                        scalar1=srb[:, oc:oc + 1])

            def stats_mu(s):
                # channel sums via (1/C)-ones matmul -> negated mean row
                xsb_s = xsb[s]
                nmr = TMP.tile([1, ST], BF16, tag="nmr", name=f"nmr{s}")
                sxp = lane_tile([1, ST], F32)
                nc.tensor.matmul(sxp[:], ones1[:], xsb_s[:, 0, :], start=True, stop=False)
                nc.tensor.matmul(sxp[:], ones1[:], xsb_s[:, 1, :], start=False, stop=True)
                nc.vector.tensor_scalar_mul(out=nmr[:], in0=sxp[:], scalar1=-1.0)
                nm2s[s] = nmr

            def stats_var(s, sq_eng):
                # E[x^2] sums; [1,512] -> [128,4] repack via PE transpose (no
                # DRAM bounce); rstd via quake rsqrt (1 newton)
                xsb_s = xsb[s]
                nmr = nm2s[s]
                sq_s = STR.tile([P, 2, ST], BF16, tag="sq", name=f"sq{s}")
                for oc in range(2):
                    if sq_eng == "act":
                        nc.scalar.activation(out=sq_s[:, oc, :],
                                             in_=xsb_s[:, oc, :], func=AF.Square)
                    else:
                        sq_eng.tensor_mul(out=sq_s[:, oc, :],
                                          in0=xsb_s[:, oc, :],
                                          in1=xsb_s[:, oc, :])
                sxxp = lane_tile([1, ST], F32)
                nc.tensor.matmul(sxxp[:], ones1[:], sq_s[:, 0, :], start=True, stop=False)
                nc.tensor.matmul(sxxp[:], ones1[:], sq_s[:, 1, :], start=False, stop=True)
                exr = TMP.tile([1, ST], BF16, tag="exr", name=f"exr{s}")
                nc.vector.tensor_copy(out=exr[:], in_=sxxp[:])
                nmT = lane_tile([P, 4, 2, 2], BF16)
                for qd in range(4):
                    nc.tensor.transpose(nmT[:, qd, 0, 0:1],
                                        nmr[:, qd * P:(qd + 1) * P],
                                        ident[0:1, 0:1])
                    nc.tensor.transpose(nmT[:, qd, 1, 0:1],
                                        exr[:, qd * P:(qd + 1) * P],
                                        ident[0:1, 0:1])
                nmS = TMP.tile([P, 4, 2, 2], BF16, tag="nmS", name=f"nmS{s}")
                nc.vector.tensor_copy(out=nmS[:], in_=nmT[:])
                mur = nmS[:, :, 0, 0]
                ex2r = nmS[:, :, 1, 0]
                nmu2 = TMP.tile([P, 4], F32, tag="nmu2")
                nc.vector.scalar_tensor_tensor(out=nmu2[:], in0=mur, scalar=-1.0,
                                               in1=mur, op0=ALU.mult, op1=ALU.mult)
                ve = TMP.tile([P, 4], F32, tag="ve")
                nc.vector.scalar_tensor_tensor(out=ve[:], in0=nmu2[:], scalar=1e-5,
                                               in1=ex2r, op0=ALU.add, op1=ALU.add)
                hsh = TMP.tile([P, 4], I32, tag="hsh")
                nc.vector.tensor_scalar(out=hsh[:], in0=ve[:].bitcast(I32), scalar1=1,
                                        scalar2=None, op0=ALU.logical_shift_right)
                nc.vector.tensor_scalar(out=hsh[:], in0=hsh[:], scalar1=-1,
                                        scalar2=MAGIC, op0=ALU.mult, op1=ALU.add)
                y0 = hsh[:].bitcast(F32)
                nt = TMP.tile([P, 4], F32, tag="nt")
                nc.vector.tensor_mul(out=nt[:], in0=y0, in1=y0)
                nc.vector.scalar_tensor_tensor(out=nt[:], in0=nt[:], scalar=-0.5,
                                               in1=ve[:], op0=ALU.mult, op1=ALU.mult)
                an_s = STR.tile([P, 4], F32, tag="an", name=f"an{s}")
                nc.vector.scalar_tensor_tensor(out=an_s[:], in0=nt[:], scalar=1.5,
                                               in1=y0, op0=ALU.add, op1=ALU.mult)
                ans[s] = an_s
                ascl_s = STR.tile([P, 4], F32, tag="ascl", name=f"ascl{s}")
                nc.vector.tensor_scalar_mul(out=ascl_s[:], in0=an_s[:],
                                            scalar1=SCALE / 16.0)
                ascl[s] = ascl_s

            def kv_chunk(s, kvoc, eng="dve"):
                # lora folded into kvwT host-side: just W.xs + mean correction
                kps = lane_tile([P, ST], F32)
                nc.tensor.matmul(kps[:], kvwT[:, 0, kvoc * P:(kvoc + 1) * P],
                                 xsb[s][:, 0, :], start=True, stop=False)
                nc.tensor.matmul(kps[:], kvwT[:, 1, kvoc * P:(kvoc + 1) * P],
                                 xsb[s][:, 1, :], start=False, stop=False)
                nc.tensor.matmul(kps[:], wg1t[:, kvoc, :], nm2s[s][:],
                                 start=False, stop=True)
                if kvoc < 2:
                    # k to fp8, then one partition-folding SBUF->SBUF DMA
                    # into the DoubleRow layout
                    if s not in kts:
                        kts[s] = ACTS.tile([16, 2, 2, 4, ST], FP8, name=f"k8{s}")
                    kf8 = TMP.tile([P, ST], FP8, tag="kf8", name=f"kf8{s}{kvoc}")
                    with nc.allow_low_precision("fp8 k for DoubleRow scores"):
                        if eng == "act":
                            nc.scalar.activation(out=kf8[:], in_=kps[:],
                                                 func=AF.Copy)
                        else:
                            nc.vector.tensor_copy(out=kf8[:], in_=kps[:])
                    keng = nc.sync if (s == 0 and kvoc == 0) else nc.gpsimd
                    keng.dma_start(
                        out=kts[s][:, kvoc].rearrange("d k hl t -> d (k hl t)"),
                        in_=kf8[:])
                    return
                if s not in vtmp:
                    vtmp[s] = STR.tile([P, 2, ST], BF16, tag="vtmp", name=f"vtmp{s}")
                dst = vtmp[s][:, kvoc - 2, :]
                if eng == "act":
                    nc.scalar.activation(out=dst, in_=kps[:], func=AF.Copy)
                else:
                    nc.vector.tensor_copy(out=dst, in_=kps[:])

            def v_transpose(s):
                # v to [m, c] (PE transpose), x rstd, + ones column;
                # scales split DVE/Pool so neither queue backs up
                vsb_s = STR.tile([P, 4, HEAD, DH + 1], BF16, tag="vsb",
                                 name=f"vsb{s}")
                for vc in range(2):
                    for u4 in range(4):
                        vtp = lane_tile([P, P], BF16)
                        nc.tensor.transpose(vtp[:],
                                            vtmp[s][:, vc, u4 * P:(u4 + 1) * P],
                                            ident[:])
                        nc.vector.tensor_scalar_mul(
                            out=vsb_s[:, u4, vc * 4:(vc + 1) * 4, 0:DH],
                            in0=vtp[:].rearrange("p (h d) -> p h d", d=DH),
                            scalar1=ans[s][:, u4:u4 + 1])
                nc.gpsimd.memset(vsb_s[:, :, :, DH:DH + 1], 1.0)
                vsb[s] = vsb_s

            def q_oc(oc):
                # q lora folded into qwT host-side; +bias straight to fp8,
                # then a DRAM bounce into the DoubleRow layout
                xs_t = xTs[0]
                qps = PSB.tile([P, NB], F32, tag="big", name=f"qps{oc}")
                for nh in range(2):
                    sl = slice(nh * 512, (nh + 1) * 512)
                    nc.tensor.matmul(qps[:, sl],
                                     qwT[:, 0, oc * P:(oc + 1) * P],
                                     xs_t[:, 0, sl], start=True, stop=False)
                    nc.tensor.matmul(qps[:, sl],
                                     qwT[:, 1, oc * P:(oc + 1) * P],
                                     xs_t[:, 1, sl], start=False, stop=True)
                qf8 = TMP.tile([P, NB], FP8, tag="qf8", name=f"qf8{oc}")
                with nc.allow_low_precision("fp8 q for DoubleRow scores"):
                    nc.vector.tensor_scalar_add(
                        out=qf8[:], in0=qps[:], scalar1=qb[:, oc:oc + 1])
                qeng = nc.sync if oc == 0 else nc.gpsimd
                qeng.dma_start(
                    out=q8[:, oc].rearrange("d k hl n -> d (k hl n)"),
                    in_=qf8[:])

            # ---------- attention pieces ----------
            pts = {}

            def emit_scores(g, mc):
                s, ml = mc // 4, mc % 4
                for h01 in range(2):
                    h = 2 * g + h01
                    stile = PSB.tile([P, NB], F32, tag="big")
                    lhsT = kts[s][:, h // 4, :, h % 4, ml * P:(ml + 1) * P]
                    for nh in range(2):
                        sl = slice(nh * 512, (nh + 1) * 512)
                        nc.tensor.matmul(stile[:, sl], lhsT,
                                         q8[:, h // 4, :, h % 4, sl],
                                         start=True, stop=True,
                                         perf_mode=DR,
                                         tile_position=(0, 0))
                    pt_t = PT.tile([P, NB], BF16, tag="pt")
                    nc.scalar.activation(out=pt_t[:], in_=stile[:],
                                         func=AF.Exp,
                                         scale=ascl[s][:, ml:ml + 1])
                    pts[(h01, mc)] = pt_t

            def pv_mm(g, mc, pvps):
                s, ml = mc // 4, mc % 4
                for nh in range(2):
                    sl = slice(nh * 512, (nh + 1) * 512)
                    for h01 in range(2):
                        h = 2 * g + h01
                        nc.tensor.matmul(
                            pvps[nh][64 * h01:64 * h01 + DH + 1, :],
                            vsb[s][:, ml, h, :], pts[(h01, mc)][:, sl],
                            start=(mc == 0), stop=(mc == 7),
                            tile_position=(0, 64 * h01))

            def pv_den(pvp2, rec2):
                # softmax denominators -> reciprocals (per nh and head band)
                ra, rb = rec2
                with nc.allow_low_precision("1/D in bf16 for the broadcast "
                                            "matmul; ~0.1% rms on the output"):
                    for nh in range(2):
                        sl = slice(nh * 512, (nh + 1) * 512)
                        nc.vector.reciprocal(out=ra[:, sl],
                                             in_=pvp2[nh][DH:DH + 1, :])
                        nc.vector.reciprocal(out=rb[:, sl],
                                             in_=pvp2[nh][64 + DH:64 + DH + 1, :])

            def pv_norm(g, nh, pvp, rec2):
                # broadcast 1/D down the partitions via mask-matmul, then
                # normalize straight into outT; the last group splits the
                # muls DVE/Pool to shorten the final drain
                ch, r0 = g // 2, 64 * (g % 2)
                sl = slice(nh * 512, (nh + 1) * 512)
                ra, rb = rec2
                recb = lane_tile([97, 512], F32)
                rsb = TMP.tile([97, 512], F32, tag="rsb", name=f"rsb{g}{nh}")
                for h01, rr in ((0, ra), (1, rb)):
                    rows = slice(64 * h01, 64 * h01 + 32)
                    nc.tensor.matmul(recb[rows, :], ones32,
                                     rr[:, sl], start=True, stop=True,
                                     tile_position=(0, 64 * h01))
                    # per-band SBUF staging so each mul starts as soon as
                    # its own broadcast lands; Act helps on the final group
                    if g == 3:
                        nc.scalar.activation(out=rsb[rows, :], in_=recb[rows, :],
                                             func=AF.Copy)
                    else:
                        nc.vector.tensor_copy(out=rsb[rows, :], in_=recb[rows, :])
                    nc.vector.tensor_mul(
                        out=outT[r0 + 32 * h01:r0 + 32 * h01 + 32, ch, sl],
                        in0=pvp[64 * h01:64 * h01 + DH, :],
                        in1=rsb[rows, :])

            def pv_tails(g):
                pv_den(pvps[g], rec2s[g])
                pv_norm(g, 0, pvps[g][0], rec2s[g])
                pv_norm(g, 1, pvps[g][1], rec2s[g])

            finbig = FIN.tile([P, 8, C], F32, tag="fin", name="finb")

            def proj(t8, ppb):
                # pp slices live in a freed score-PSUM buffer; the proj bias
                # is a rank-1 ones x pb matmul accumulated into the same
                # PSUM; the (idle) Act engine stages PSUM->SBUF for the DMA
                pp = ppb[:, t8 % 4, :]
                nc.tensor.matmul(pp, outT[:, 0, t8 * P:(t8 + 1) * P],
                                 pwT[:, 0, :], start=True, stop=False)
                nc.tensor.matmul(pp, outT[:, 1, t8 * P:(t8 + 1) * P],
                                 pwT[:, 1, :], start=False, stop=False)
                nc.tensor.matmul(pp, onesb, pbb, start=False, stop=True)
                fin = finbig[:, t8, :]
                if t8 % 2 == 0:
                    nc.scalar.activation(out=fin, in_=pp, func=AF.Copy)
                else:
                    nc.vector.tensor_copy(out=fin, in_=pp)
                if t8 % 2 == 1:
                    nc.sync.dma_start(
                        out=out_d[(t8 - 1) * P:(t8 + 1) * P, :].rearrange(
                            "(t p) c -> p t c", t=2),
                        in_=finbig[:, t8 - 1:t8 + 1, :])

            # ---------- emission schedule ----------
            # preamble: shortest chain to the first score tile — q lora,
            # conv(0), mean, q(oc0), shared lora, k chunk 0, then variance
            # (only needed for the exp scale / v scales, off the PE path)
            conv(0, cc_major=True)
            stats_mu(0)
            kv_chunk(0, 0, eng="act")
            q_oc(0)
            stats_var(0, "act")

            seq = [(g, mc) for g in range(4) for mc in range(8)]
            pvps = {}
            rec2s = {}

            def ensure_group(g):
                if g not in pvps:
                    pvps[g] = (PSV.tile([P, 512], F32, tag="pv", name=f"pv{g}a"),
                               PSV.tile([P, 512], F32, tag="pv", name=f"pv{g}b"))
                    rec2s[g] = (ATN.tile([1, NB], BF16, tag="reca", name=f"reca{g}"),
                                ATN.tile([1, NB], BF16, tag="recb", name=f"recb{g}"))

            def s1_kv0():
                stats_mu(1)
                kv_chunk(1, 0, eng="dve")

            # strip-1 setup interleaved between early attention steps; each
            # filler runs right AFTER that step's score matmuls, ordered so
            # every tile exists by the emission that references it and no
            # burst outruns the ~2us cushion the score double-buffer gives
            filler = [lambda: conv(1, (0,)),
                      lambda: conv(1, (1,)),
                      s1_kv0,
                      lambda: stats_var(1, nc.vector),
                      lambda: (kv_chunk(0, 2), kv_chunk(0, 3), v_transpose(0)),
                      lambda: (kv_chunk(0, 1), kv_chunk(1, 1)),
                      lambda: (kv_chunk(1, 2), kv_chunk(1, 3)),
                      lambda: v_transpose(1),
                      lambda: q_oc(1)]
            fi = 0

            # pv trails scores by 4 so group-boundary chains stay hidden;
            # the last group trails by only 2 so the final drain is short
            def lag_of(k):
                return 2 if k >= 24 else 4

            next_pv = 0
            for i, (g, mc) in enumerate(seq):
                ensure_group(g)
                emit_scores(g, mc)
                if fi < len(filler):
                    filler[fi]()
                    fi += 1
                while next_pv <= i - lag_of(next_pv):
                    pg, pmc = seq[next_pv]
                    pv_mm(pg, pmc, pvps[pg])
                    next_pv += 1
                    if pmc == 7:
                        pv_tails(pg)
            while next_pv < 32:
                pg, pmc = seq[next_pv]
                pv_mm(pg, pmc, pvps[pg])
                next_pv += 1
            pv_tails(3)
            ppA = PSB.tile([P, 4, C], F32, tag="big", name="ppA")
            ppB = PSB.tile([P, 4, C], F32, tag="big", name="ppB")
            for t8 in range(8):
                proj(t8, ppA if t8 < 4 else ppB)

    nc.finalize()
    return nc


def _prep_shared(q_w, q_b, kv_w, kv_b, proj_w, proj_b, a_q, b_q, a_v, b_v,
                 sr_w, sr_b, ln_g, ln_b):
    f32 = np.float32

    def chunkT(w):  # [in, out] -> [128, n_in_chunks, out]
        wt = np.ascontiguousarray(np.asarray(w, f32).T)
        ic, oc = wt.shape
        return np.ascontiguousarray(
            wt.reshape(ic // 128, 128, oc).transpose(1, 0, 2)).astype(BF16NP)

    def pcols(v):  # [n*128] -> [128, n]
        v = np.asarray(v, f32)
        return np.ascontiguousarray(v.reshape(-1, 128).T)

    kv_w = np.asarray(kv_w, f32)
    a_v = np.asarray(a_v, f32)
    b_v = np.asarray(b_v, f32)
    g = np.asarray(ln_g, f32)
    bb = np.asarray(ln_b, f32)
    proj_w = np.asarray(proj_w, f32)
    # exact host-side folds: both LoRA adapters act on the same input as
    # their base projection, so W_eff = W + B@A; then fold LayerNorm gamma
    # into kv weights (mean via rank-1 correction), drop k-side constants
    # (softmax shift invariance), fold v-side constants into the proj bias,
    # and fold the softmax 1/sqrt(dh) into every k-side weight
    qw_eff = (np.asarray(q_w, f32)
              + np.asarray(b_q, f32) @ np.asarray(a_q, f32)) * 4.0
    delta_w = b_v @ a_v            # shared lora delta applied to both k and v
    W_all = kv_w.copy()
    W_all[:C] += delta_w
    W_all[C:] += delta_w
    Wg = W_all * g[None, :]
    wg1 = Wg.sum(1)
    wbt = W_all @ bb + np.asarray(kv_b, f32)
    pb_eff = np.asarray(proj_b, f32) + proj_w @ wbt[C:]

    # permutation: psum row r = d*8 + kt*4 + hl  <-  channel hl*32 + kt*16 + d
    rr = np.arange(128)
    perm = (rr % 4) * 32 + ((rr % 8) // 4) * 16 + rr // 8

    srwT = np.asarray(sr_w, f32).transpose(1, 2, 3, 0).reshape(2, 128, 4, C)
    srwT = np.ascontiguousarray(srwT.transpose(1, 0, 2, 3)).astype(BF16NP)

    # q and k each scaled x4 so fp8e4m3 stays out of subnormals; the
    # softmax scale divides the 16x back out
    Wgs = Wg.copy()
    Wgs[:C, :] *= 4.0
    wg1s = wg1.copy()
    wg1s[:C] *= 4.0
    wA = srwT.reshape(128, 2048)
    qwTa = chunkT(qw_eff)                       # [128, 2, 512]
    qwTa = qwTa.reshape(128, 2, 2, 128)[:, :, :, perm].reshape(128, 512)
    wB1 = np.ascontiguousarray(qwTa)
    wB2 = np.zeros((128, 2432), BF16NP)
    kvwTa = chunkT(Wgs).reshape(128, 2, 4, 128)
    kvwTa[:, :, 0:2, :] = kvwTa[:, :, 0:2, perm]
    wB2[:, 0:1024] = kvwTa.reshape(128, 1024)
    wB2[:, 1024:1536] = chunkT(proj_w).reshape(128, 512)
    wg1a = wg1s.reshape(4, 128).copy()
    wg1a[0:2, :] = wg1a[0:2, perm]
    wB2[0:1, 1536:2048] = wg1a.astype(BF16NP).reshape(1, 512)
    wB2[0, 2048:2176] = BF16NP(1.0)   # ones row: proj-bias + 1/D broadcast
    wB2[0, 2176:2432] = pb_eff.astype(BF16NP)
    wC = np.zeros((128, 4), f32)
    wC[:, 0:2] = pcols(q_b)[perm] * 4.0
    wC[:, 2:4] = pcols(sr_b)
    return dict(wA=np.ascontiguousarray(wA), wB1=wB1, wB2=wB2, wC=wC)


def kernel(x, q_w, q_b, kv_w, kv_b, proj_w, proj_b, a_q, b_q, a_v, b_v,
           sr_w, sr_b, ln_g, ln_b, H, W):
    from concourse.bass_utils import run_bass_kernel_spmd

    x = np.asarray(x, np.float32)
    assert x.shape == (B, N, C) and int(H) == 64 and int(W) == 64

    if "nc" not in _CACHE:
        _CACHE["nc"] = _build_program()
    nc = _CACHE["nc"]

    shared = _prep_shared(q_w, q_b, kv_w, kv_b, proj_w, proj_b, a_q, b_q,
                          a_v, b_v, sr_w, sr_b, ln_g, ln_b)
    in_maps = []
    for c in range(NCORES):
        b, j = c // 4, c % 4
        xb = np.roll(x[b], -NB * j, axis=0)          # own block at rows 0:1024
        xT = np.ascontiguousarray(xb.T.astype(BF16NP))  # [256, 4096]
        xT = np.ascontiguousarray(
            xT.reshape(2, 128, N).transpose(1, 0, 2))   # [128, 2, 4096]
        in_maps.append(dict(shared, xT=xT))

    res = run_bass_kernel_spmd(nc, in_maps, list(range(NCORES)))
    out = np.empty((B, N, C), np.float32)
    for c in range(NCORES):
        b, j = c // 4, c % 4
        out[b, NB * j:NB * (j + 1)] = res.results[c]["out"]
    return out


# revision 67
# speedup vs baseline: 1.0131x; 1.0131x over previous
"""Trainium2 Bass kernel for PVT-style spatial-reduction attention.

Shapes (hardcoded): x [2, 4096, 256], HEAD=8, dh=32, SR=2, R=8, H=W=64.
Sharding: core c = (batch b = c//4, query block j = c%4). Each core computes
q/attention/proj for its 1024 query rows and redundantly computes the small
conv+LN+KV path for its batch (no collectives). Per-core x is pre-rotated on
host so each core's own query block is rows 0:1024.

Structure (driven by the TimelineSim cost model; 147us baseline -> 101us):
- The Activation engine's exp stream (64 x [128,1024] tiles ~= 66us) is the
  hard floor; everything else is arranged to keep it 100% fed (total exp
  stalls ~1.5us).
- Score matmuls run in fp8e4m3 DoubleRow (half cost, PE gains the slack
  that previously starved the exp stream). q/k are scaled x4 host-side to
  stay out of fp8 subnormals; the exp scale divides the 16x back out with
  the per-token LayerNorm rstd. Weight output columns are host-permuted so
  the q/k PSUM rows come out (dlo, ktile, head)-ordered and one partition-
  folding SBUF->SBUF DMA produces the [16, 2, ...] DoubleRow layout.
- Both LoRA adapters are folded into their base weights host-side
  (W + B@A, exact); LayerNorm gamma/mean fold into the kv weights, v-side
  constants into the proj bias (rank-1 ones-row matmul).
- Weights packed into a few blob DMAs (HWDGE serializes ~1.3us per DMA),
  ordered so conv's inputs land first; PE warmup matmuls during the DMA
  phase hold the p-state at full clock for conv.
- Strip-0 attention starts as soon as strip-0 K + Q exist; strip-1
  conv/LN/KV is interleaved into the PE queue between early score tiles.
- LN-stat repack and softmax-denominator broadcast use PE transposes /
  ones-matmuls instead of DRAM round-trips; normalized outputs are written
  by the DVE directly into the outT layout (partition rebase, no DMA).
- Attention is software-pipelined: pv trails scores by 4 steps (2 for the
  last group) so group-boundary denominator chains hide under the exp
  cushion; the output projection/staging splits across PE/Act/DVE with
  paired output DMAs.

PSUM budget (8 banks): scores 2x[128,1024] (4) + pv 2x[128,512] (2) +
lane A (1) + lane B (1); lanes A/B carry conv/stats/kv/vT in setup and
recb/proj afterwards.
"""
import sys

if "/opt/trn_rl_repo" not in sys.path:
    sys.path.insert(0, "/opt/trn_rl_repo")

import numpy as np
import ml_dtypes

BF16NP = ml_dtypes.bfloat16

HEAD, DH, C, N, B, M, R = 8, 32, 256, 4096, 2, 1024, 8
NB = N // 4          # query rows per core
SCALE = DH ** -0.5
NCORES = 8
MAGIC = 0x5F3759DF

_CACHE = {}


def _build_program():
    import concourse.bass as bass
    import concourse.tile as tile
    from concourse.bacc import Bacc
    from concourse import mybir, masks

    F32 = mybir.dt.float32
    F32R = mybir.dt.float32r
    FP8 = mybir.dt.float8e4
    DR = mybir.MatmulPerfMode.DoubleRow
    BF16 = mybir.dt.bfloat16
    I32 = mybir.dt.int32
    AF = mybir.ActivationFunctionType
    ALU = mybir.AluOpType

    nc = Bacc()
    P = 128
    S = 2          # kv strips
    ST = 512       # kv tokens per strip

    # ---- DRAM parameters (host-prepped packed layouts) ----
    xT_d = nc.declare_dram_parameter("xT", [P, 2, N], BF16, isOutput=False)
    wA_d = nc.declare_dram_parameter("wA", [P, 2048], BF16, isOutput=False)
    wB1_d = nc.declare_dram_parameter("wB1", [P, 512], BF16, isOutput=False)
    wB2_d = nc.declare_dram_parameter("wB2", [P, 2432], BF16, isOutput=False)
    wC_d = nc.declare_dram_parameter("wC", [P, 4], F32, isOutput=False)
    out_d = nc.declare_dram_parameter("out", [NB, C], F32, isOutput=True)

    with tile.TileContext(nc) as tc:
        with tc.tile_pool(name="wgt", bufs=1) as WGT, \
             tc.tile_pool(name="acts", bufs=1) as ACTS, \
             tc.tile_pool(name="strips", bufs=2) as STR, \
             tc.tile_pool(name="tmp", bufs=3) as TMP, \
             tc.tile_pool(name="atn", bufs=2) as ATN, \
             tc.tile_pool(name="pt", bufs=32) as PT, \
             tc.tile_pool(name="fin", bufs=2) as FIN, \
             tc.tile_pool(name="big", bufs=2, space="PSUM") as PSB, \
             tc.tile_pool(name="pv", bufs=2, space="PSUM") as PSV, \
             tc.tile_pool(name="la", bufs=1, space="PSUM") as PSC, \
             tc.tile_pool(name="lb", bufs=1, space="PSUM") as PSK:

            # ---------- warmups + loads ----------
            # dummy exp pulls the Exp ACT_TABLE_LOAD off the critical path
            warm = WGT.tile([1, 2], F32, tag="warm")
            nc.gpsimd.memset(warm[:], 0.0)
            warmo = WGT.tile([1, 2], F32, tag="warmo")
            nc.scalar.activation(out=warmo[:], in_=warm[:], func=AF.Exp)
            # matmul fodder to ramp the PE p-state during the DMA phase
            wz = WGT.tile([P, 256], BF16, tag="wz")
            nc.gpsimd.memset(wz[:], 0.0)

            wA = WGT.tile([P, 2048], BF16, tag="wA")
            nc.sync.dma_start(out=wA[:], in_=wA_d[:])
            xs0 = ACTS.tile([P, 2, 2048], BF16, tag="xT0")
            nc.sync.dma_start(out=xs0[:, 0, :], in_=xT_d[:, 0, 0:2048])
            nc.sync.dma_start(out=xs0[:, 1, :], in_=xT_d[:, 1, 0:2048])
            wB1 = WGT.tile([P, 512], BF16, tag="wB1")
            nc.sync.dma_start(out=wB1[:], in_=wB1_d[:])
            wC = WGT.tile([P, 4], F32, tag="wC")
            nc.sync.dma_start(out=wC[:], in_=wC_d[:])
            wB2 = WGT.tile([P, 2432], BF16, tag="wB2")
            nc.sync.dma_start(out=wB2[:], in_=wB2_d[:])
            xs1 = ACTS.tile([P, 2, 2048], BF16, tag="xT1")
            nc.sync.dma_start(out=xs1[:], in_=xT_d[:, :, 2048:4096])
            xTs = [xs0, xs1]

            srwT = wA[:, 0:2048].rearrange("p (c f k) -> p c f k", c=2, f=4)
            qwT = wB1[:, 0:512].rearrange("p (c k) -> p c k", c=2)
            kvwT = wB2[:, 0:1024].rearrange("p (c k) -> p c k", c=2)
            pwT = wB2[:, 1024:1536].rearrange("p (c k) -> p c k", c=2)
            wg1t = wB2[0:1, 1536:2048].rearrange("p (g k) -> p g k", g=4)
            onesb = wB2[0:1, 2048:2176]
            pbb = wB2[0:1, 2176:2432]
            ones32 = onesb[0:1, 0:32]
            qb = wC[:, 0:2]
            srb = wC[:, 2:4]

            ones1 = WGT.tile([P, 1], BF16, tag="ones1")
            nc.gpsimd.memset(ones1[:], 1.0 / C)
            ident = WGT.tile([P, P], BF16, tag="ident")
            masks.make_identity(nc, ident[:])

            # keep the PE continuously busy through the DMA phase so the
            # p-state ramp (full speed after 3us busy) is done before conv
            for _ in range(16):
                wp = PSC.tile([1, 256], F32, tag="la", name="wp")
                nc.tensor.matmul(wp[:], wz[:, 0:1], wz[:], start=True, stop=True)

            # persistent activations; q/k live in the fp8 DoubleRow layout
            # [16 (dh lo), chunk, 2 (dh k-tile), 4 (head in chunk), tokens].
            # The projection weight columns are host-permuted so the PSUM
            # rows come out (d, kt, hl)-ordered and a single SBUF->SBUF DMA
            # folds 128 partitions into 16 x 8.
            q8 = ACTS.tile([16, 2, 2, 4, NB], FP8, tag="q8")
            outT = ACTS.tile([P, 2, NB], BF16, tag="outT")

            xsb, nm2s, kts, vtmp, vsb, ans, ascl = {}, {}, {}, {}, {}, {}, {}
            lane = [0]

            def lane_tile(shape, dt):
                pool = PSC if lane[0] % 2 == 0 else PSK
                tg = "la" if pool is PSC else "lb"
                lane[0] += 1
                return pool.tile(shape, dt, tag=tg, name=f"ln{lane[0]}")

            # ---------- per-strip setup pieces ----------
            def conv(s, ocs=(0, 1), cc_major=False):
                # 2x2 stride-2 conv as 8 accumulated matmuls per out-chunk;
                # +srb fold straight to bf16
                xs_t = xTs[s]
                if s not in xsb:
                    xsb[s] = STR.tile([P, 2, ST], BF16, tag="xsb", name=f"xsb{s}")
                xsb_s = xsb[s]
                cps = {oc: lane_tile([P, ST], F32) for oc in ocs}
                order = ([(cc, oc, di, dj) for cc in range(2) for oc in ocs
                          for di in range(2) for dj in range(2)]
                         if cc_major else
                         [(cc, oc, di, dj) for oc in ocs for cc in range(2)
                          for di in range(2) for dj in range(2)])
                seen = {}
                for cc, oc, di, dj in order:
                    xv = xs_t[:, cc, :].rearrange(
                        "p (i a j b) -> p i a j b", i=16, a=2, j=32, b=2)
                    k = seen.get(oc, 0)
                    seen[oc] = k + 1
                    nc.tensor.matmul(
                        cps[oc][:], srwT[:, cc, di * 2 + dj,
                                         oc * P:(oc + 1) * P],
                        xv[:, :, di, :, dj],
                        start=(k == 0), stop=(k == 7))
                for oc in ocs:
                    nc.vector.tensor_scalar_add(
                        out=xsb_s[:, oc, :], in0=cps[oc][:],
                        scalar1=srb[:, oc:oc + 1])

            def stats_mu(s):
                # channel sums via (1/C)-ones matmul -> negated mean row
                xsb_s = xsb[s]
                nmr = TMP.tile([1, ST], BF16, tag="nmr", name=f"nmr{s}")
                sxp = lane_tile([1, ST], F32)
                nc.tensor.matmul(sxp[:], ones1[:], xsb_s[:, 0, :], start=True, stop=False)
                nc.tensor.matmul(sxp[:], ones1[:], xsb_s[:, 1, :], start=False, stop=True)
                nc.vector.tensor_scalar_mul(out=nmr[:], in0=sxp[:], scalar1=-1.0)
                nm2s[s] = nmr

            def stats_var(s, sq_eng):
                # E[x^2] sums; [1,512] -> [128,4] repack via PE transpose (no
                # DRAM bounce); rstd via quake rsqrt (1 newton)
                xsb_s = xsb[s]
                nmr = nm2s[s]
                sq_s = STR.tile([P, 2, ST], BF16, tag="sq", name=f"sq{s}")
                for oc in range(2):
                    if sq_eng == "act":
                        nc.scalar.activation(out=sq_s[:, oc, :],
                                             in_=xsb_s[:, oc, :], func=AF.Square)
                    else:
                        sq_eng.tensor_mul(out=sq_s[:, oc, :],
                                          in0=xsb_s[:, oc, :],
                                          in1=xsb_s[:, oc, :])
                sxxp = lane_tile([1, ST], F32)
                nc.tensor.matmul(sxxp[:], ones1[:], sq_s[:, 0, :], start=True, stop=False)
                nc.tensor.matmul(sxxp[:], ones1[:], sq_s[:, 1, :], start=False, stop=True)
                exr = TMP.tile([1, ST], BF16, tag="exr", name=f"exr{s}")
                nc.vector.tensor_copy(out=exr[:], in_=sxxp[:])
                nmT = lane_tile([P, 4, 2, 2], BF16)
                for qd in range(4):
                    nc.tensor.transpose(nmT[:, qd, 0, 0:1],
                                        nmr[:, qd * P:(qd + 1) * P],
                                        ident[0:1, 0:1])
                    nc.tensor.transpose(nmT[:, qd, 1, 0:1],
                                        exr[:, qd * P:(qd + 1) * P],
                                        ident[0:1, 0:1])
                nmS = TMP.tile([P, 4, 2, 2], BF16, tag="nmS", name=f"nmS{s}")
                nc.vector.tensor_copy(out=nmS[:], in_=nmT[:])
                mur = nmS[:, :, 0, 0]
                ex2r = nmS[:, :, 1, 0]
                nmu2 = TMP.tile([P, 4], F32, tag="nmu2")
                nc.vector.scalar_tensor_tensor(out=nmu2[:], in0=mur, scalar=-1.0,
                                               in1=mur, op0=ALU.mult, op1=ALU.mult)
                ve = TMP.tile([P, 4], F32, tag="ve")
                nc.vector.scalar_tensor_tensor(out=ve[:], in0=nmu2[:], scalar=1e-5,
                                               in1=ex2r, op0=ALU.add, op1=ALU.add)
                hsh = TMP.tile([P, 4], I32, tag="hsh")
                nc.vector.tensor_scalar(out=hsh[:], in0=ve[:].bitcast(I32), scalar1=1,
                                        scalar2=None, op0=ALU.logical_shift_right)
                nc.vector.tensor_scalar(out=hsh[:], in0=hsh[:], scalar1=-1,
                                        scalar2=MAGIC, op0=ALU.mult, op1=ALU.add)
                y0 = hsh[:].bitcast(F32)
                nt = TMP.tile([P, 4], F32, tag="nt")
                nc.vector.tensor_mul(out=nt[:], in0=y0, in1=y0)
                nc.vector.scalar_tensor_tensor(out=nt[:], in0=nt[:], scalar=-0.5,
                                               in1=ve[:], op0=ALU.mult, op1=ALU.mult)
                an_s = STR.tile([P, 4], F32, tag="an", name=f"an{s}")
                nc.vector.scalar_tensor_tensor(out=an_s[:], in0=nt[:], scalar=1.5,
                                               in1=y0, op0=ALU.add, op1=ALU.mult)
                ans[s] = an_s
                ascl_s = STR.tile([P, 4], F32, tag="ascl", name=f"ascl{s}")
                nc.vector.tensor_scalar_mul(out=ascl_s[:], in0=an_s[:],
                                            scalar1=SCALE / 16.0)
                ascl[s] = ascl_s

            def kv_chunk(s, kvoc, eng="dve"):
                # lora folded into kvwT host-side: just W.xs + mean correction
                kps = lane_tile([P, ST], F32)
                nc.tensor.matmul(kps[:], kvwT[:, 0, kvoc * P:(kvoc + 1) * P],
                                 xsb[s][:, 0, :], start=True, stop=False)
                nc.tensor.matmul(kps[:], kvwT[:, 1, kvoc * P:(kvoc + 1) * P],
                                 xsb[s][:, 1, :], start=False, stop=False)
                nc.tensor.matmul(kps[:], wg1t[:, kvoc, :], nm2s[s][:],
                                 start=False, stop=True)
                if kvoc < 2:
                    # k to fp8, then one partition-folding SBUF->SBUF DMA
                    # into the DoubleRow layout
                    if s not in kts:
                        kts[s] = ACTS.tile([16, 2, 2, 4, ST], FP8, name=f"k8{s}")
                    kf8 = TMP.tile([P, ST], FP8, tag="kf8", name=f"kf8{s}{kvoc}")
                    with nc.allow_low_precision("fp8 k for DoubleRow scores"):
                        if eng == "act":
                            nc.scalar.activation(out=kf8[:], in_=kps[:],
                                                 func=AF.Copy)
                        else:
                            nc.vector.tensor_copy(out=kf8[:], in_=kps[:])
                    keng = nc.sync if (s == 0 and kvoc == 0) else nc.gpsimd
                    keng.dma_start(
                        out=kts[s][:, kvoc].rearrange("d k hl t -> d (k hl t)"),
                        in_=kf8[:])
                    return
                if s not in vtmp:
                    vtmp[s] = STR.tile([P, 2, ST], BF16, tag="vtmp", name=f"vtmp{s}")
                dst = vtmp[s][:, kvoc - 2, :]
                if eng == "act":
                    nc.scalar.activation(out=dst, in_=kps[:], func=AF.Copy)
                else:
                    nc.vector.tensor_copy(out=dst, in_=kps[:])

            def v_transpose(s):
                # v to [m, c] (PE transpose), x rstd, + ones column;
                # scales split DVE/Pool so neither queue backs up
                vsb_s = STR.tile([P, 4, HEAD, DH + 1], BF16, tag="vsb",
                                 name=f"vsb{s}")
                for vc in range(2):
                    for u4 in range(4):
                        vtp = lane_tile([P, P], BF16)
                        nc.tensor.transpose(vtp[:],
                                            vtmp[s][:, vc, u4 * P:(u4 + 1) * P],
                                            ident[:])
                        nc.vector.tensor_scalar_mul(
                            out=vsb_s[:, u4, vc * 4:(vc + 1) * 4, 0:DH],
                            in0=vtp[:].rearrange("p (h d) -> p h d", d=DH),
                            scalar1=ans[s][:, u4:u4 + 1])
                nc.gpsimd.memset(vsb_s[:, :, :, DH:DH + 1], 1.0)
                vsb[s] = vsb_s

            def q_oc(oc):
                # q lora folded into qwT host-side; +bias straight to fp8,
                # then a DRAM bounce into the DoubleRow layout
                xs_t = xTs[0]
                qps = PSB.tile([P, NB], F32, tag="big", name=f"qps{oc}")
                for nh in range(2):
                    sl = slice(nh * 512, (nh + 1) * 512)
                    nc.tensor.matmul(qps[:, sl],
                                     qwT[:, 0, oc * P:(oc + 1) * P],
                                     xs_t[:, 0, sl], start=True, stop=False)
                    nc.tensor.matmul(qps[:, sl],
                                     qwT[:, 1, oc * P:(oc + 1) * P],
                                     xs_t[:, 1, sl], start=False, stop=True)
                qf8 = TMP.tile([P, NB], FP8, tag="qf8", name=f"qf8{oc}")
                with nc.allow_low_precision("fp8 q for DoubleRow scores"):
                    nc.vector.tensor_scalar_add(
                        out=qf8[:], in0=qps[:], scalar1=qb[:, oc:oc + 1])
                nc.gpsimd.dma_start(
                    out=q8[:, oc].rearrange("d k hl n -> d (k hl n)"),
                    in_=qf8[:])

            # ---------- attention pieces ----------
            pts = {}

            def emit_scores(g, mc):
                s, ml = mc // 4, mc % 4
                for h01 in range(2):
                    h = 2 * g + h01
                    stile = PSB.tile([P, NB], F32, tag="big")
                    lhsT = kts[s][:, h // 4, :, h % 4, ml * P:(ml + 1) * P]
                    for nh in range(2):
                        sl = slice(nh * 512, (nh + 1) * 512)
                        nc.tensor.matmul(stile[:, sl], lhsT,
                                         q8[:, h // 4, :, h % 4, sl],
                                         start=True, stop=True,
                                         perf_mode=DR,
                                         tile_position=(0, 0))
                    pt_t = PT.tile([P, NB], BF16, tag="pt")
                    nc.scalar.activation(out=pt_t[:], in_=stile[:],
                                         func=AF.Exp,
                                         scale=ascl[s][:, ml:ml + 1])
                    pts[(h01, mc)] = pt_t

            def pv_mm(g, mc, pvps):
                s, ml = mc // 4, mc % 4
                for nh in range(2):
                    sl = slice(nh * 512, (nh + 1) * 512)
                    for h01 in range(2):
                        h = 2 * g + h01
                        nc.tensor.matmul(
                            pvps[nh][64 * h01:64 * h01 + DH + 1, :],
                            vsb[s][:, ml, h, :], pts[(h01, mc)][:, sl],
                            start=(mc == 0), stop=(mc == 7),
                            tile_position=(0, 64 * h01))

            def pv_den(pvp2, rec2):
                # softmax denominators -> reciprocals (per nh and head band)
                ra, rb = rec2
                with nc.allow_low_precision("1/D in bf16 for the broadcast "
                                            "matmul; ~0.1% rms on the output"):
                    for nh in range(2):
                        sl = slice(nh * 512, (nh + 1) * 512)
                        nc.vector.reciprocal(out=ra[:, sl],
                                             in_=pvp2[nh][DH:DH + 1, :])
                        nc.vector.reciprocal(out=rb[:, sl],
                                             in_=pvp2[nh][64 + DH:64 + DH + 1, :])

            def pv_norm(g, nh, pvp, rec2):
                # broadcast 1/D down the partitions via mask-matmul, then
                # normalize straight into outT; the last group splits the
                # muls DVE/Pool to shorten the final drain
                ch, r0 = g // 2, 64 * (g % 2)
                sl = slice(nh * 512, (nh + 1) * 512)
                ra, rb = rec2
                recb = lane_tile([97, 512], F32)
                rsb = TMP.tile([97, 512], F32, tag="rsb", name=f"rsb{g}{nh}")
                for h01, rr in ((0, ra), (1, rb)):
                    rows = slice(64 * h01, 64 * h01 + 32)
                    nc.tensor.matmul(recb[rows, :], ones32,
                                     rr[:, sl], start=True, stop=True,
                                     tile_position=(0, 64 * h01))
                    # per-band SBUF staging so each mul starts as soon as
                    # its own broadcast lands; Act helps on the final group
                    if g == 3:
                        nc.scalar.activation(out=rsb[rows, :], in_=recb[rows, :],
                                             func=AF.Copy)
                    else:
                        nc.vector.tensor_copy(out=rsb[rows, :], in_=recb[rows, :])
                    nc.vector.tensor_mul(
                        out=outT[r0 + 32 * h01:r0 + 32 * h01 + 32, ch, sl],
                        in0=pvp[64 * h01:64 * h01 + DH, :],
                        in1=rsb[rows, :])

            def pv_tails(g):
                pv_den(pvps[g], rec2s[g])
                pv_norm(g, 0, pvps[g][0], rec2s[g])
                pv_norm(g, 1, pvps[g][1], rec2s[g])

            finbig = FIN.tile([P, 8, C], F32, tag="fin", name="finb")

            def proj(t8, ppb):
                # pp slices live in a freed score-PSUM buffer; the proj bias
                # is a rank-1 ones x pb matmul accumulated into the same
                # PSUM; the (idle) Act engine stages PSUM->SBUF for the DMA
                pp = ppb[:, t8 % 4, :]
                nc.tensor.matmul(pp, outT[:, 0, t8 * P:(t8 + 1) * P],
                                 pwT[:, 0, :], start=True, stop=False)
                nc.tensor.matmul(pp, outT[:, 1, t8 * P:(t8 + 1) * P],
                                 pwT[:, 1, :], start=False, stop=False)
                nc.tensor.matmul(pp, onesb, pbb, start=False, stop=True)
                fin = finbig[:, t8, :]
                if t8 % 2 == 0:
                    nc.scalar.activation(out=fin, in_=pp, func=AF.Copy)
                else:
                    nc.vector.tensor_copy(out=fin, in_=pp)
                if t8 % 2 == 1:
                    nc.sync.dma_start(
                        out=out_d[(t8 - 1) * P:(t8 + 1) * P, :].rearrange(
                            "(t p) c -> p t c", t=2),
                        in_=finbig[:, t8 - 1:t8 + 1, :])

            # ---------- emission schedule ----------
            # preamble: shortest chain to the first score tile — q lora,
            # conv(0), mean, q(oc0), shared lora, k chunk 0, then variance
            # (only needed for the exp scale / v scales, off the PE path)
            conv(0, cc_major=True)
            stats_mu(0)
            kv_chunk(0, 0, eng="act")
            q_oc(0)
            stats_var(0, "act")

            seq = [(g, mc) for g in range(4) for mc in range(8)]
            pvps = {}
            rec2s = {}

            def ensure_group(g):
                if g not in pvps:
                    pvps[g] = (PSV.tile([P, 512], F32, tag="pv", name=f"pv{g}a"),
                               PSV.tile([P, 512], F32, tag="pv", name=f"pv{g}b"))
                    rec2s[g] = (ATN.tile([1, NB], BF16, tag="reca", name=f"reca{g}"),
                                ATN.tile([1, NB], BF16, tag="recb", name=f"recb{g}"))

            def s1_kv0():
                stats_mu(1)
                kv_chunk(1, 0, eng="dve")

            # strip-1 setup interleaved between early attention steps; each
            # filler runs right AFTER that step's score matmuls, ordered so
            # every tile exists by the emission that references it and no
            # burst outruns the ~2us cushion the score double-buffer gives
            filler = [lambda: conv(1, (0,)),
                      lambda: conv(1, (1,)),
                      s1_kv0,
                      lambda: stats_var(1, nc.vector),
                      lambda: (kv_chunk(0, 2), kv_chunk(0, 3), v_transpose(0)),
                      lambda: (kv_chunk(0, 1), kv_chunk(1, 1)),
                      lambda: (kv_chunk(1, 2), kv_chunk(1, 3)),
                      lambda: v_transpose(1),
                      lambda: q_oc(1)]
            fi = 0

            # pv trails scores by 4 so group-boundary chains stay hidden;
            # the last group trails by only 2 so the final drain is short
            def lag_of(k):
                return 2 if k >= 24 else 4

            next_pv = 0
            for i, (g, mc) in enumerate(seq):
                ensure_group(g)
                emit_scores(g, mc)
                if fi < len(filler):
                    filler[fi]()
                    fi += 1
                while next_pv <= i - lag_of(next_pv):
                    pg, pmc = seq[next_pv]
                    pv_mm(pg, pmc, pvps[pg])
                    next_pv += 1
                    if pmc == 7:
                        pv_tails(pg)
            while next_pv < 32:
                pg, pmc = seq[next_pv]
                pv_mm(pg, pmc, pvps[pg])
                next_pv += 1
            pv_tails(3)
            ppA = PSB.tile([P, 4, C], F32, tag="big", name="ppA")
            ppB = PSB.tile([P, 4, C], F32, tag="big", name="ppB")
            for t8 in range(8):
                proj(t8, ppA if t8 < 4 else ppB)

    nc.finalize()
    return nc


def _prep_shared(q_w, q_b, kv_w, kv_b, proj_w, proj_b, a_q, b_q, a_v, b_v,
                 sr_w, sr_b, ln_g, ln_b):
    f32 = np.float32

    def chunkT(w):  # [in, out] -> [128, n_in_chunks, out]
        wt = np.ascontiguousarray(np.asarray(w, f32).T)
        ic, oc = wt.shape
        return np.ascontiguousarray(
            wt.reshape(ic // 128, 128, oc).transpose(1, 0, 2)).astype(BF16NP)

    def pcols(v):  # [n*128] -> [128, n]
        v = np.asarray(v, f32)
        return np.ascontiguousarray(v.reshape(-1, 128).T)

    kv_w = np.asarray(kv_w, f32)
    a_v = np.asarray(a_v, f32)
    b_v = np.asarray(b_v, f32)
    g = np.asarray(ln_g, f32)
    bb = np.asarray(ln_b, f32)
    proj_w = np.asarray(proj_w, f32)
    # exact host-side folds: both LoRA adapters act on the same input as
    # their base projection, so W_eff = W + B@A; then fold LayerNorm gamma
    # into kv weights (mean via rank-1 correction), drop k-side constants
    # (softmax shift invariance), fold v-side constants into the proj bias,
    # and fold the softmax 1/sqrt(dh) into every k-side weight
    qw_eff = (np.asarray(q_w, f32)
              + np.asarray(b_q, f32) @ np.asarray(a_q, f32)) * 4.0
    delta_w = b_v @ a_v            # shared lora delta applied to both k and v
    W_all = kv_w.copy()
    W_all[:C] += delta_w
    W_all[C:] += delta_w
    Wg = W_all * g[None, :]
    wg1 = Wg.sum(1)
    wbt = W_all @ bb + np.asarray(kv_b, f32)
    pb_eff = np.asarray(proj_b, f32) + proj_w @ wbt[C:]

    # permutation: psum row r = d*8 + kt*4 + hl  <-  channel hl*32 + kt*16 + d
    rr = np.arange(128)
    perm = (rr % 4) * 32 + ((rr % 8) // 4) * 16 + rr // 8

    srwT = np.asarray(sr_w, f32).transpose(1, 2, 3, 0).reshape(2, 128, 4, C)
    srwT = np.ascontiguousarray(srwT.transpose(1, 0, 2, 3)).astype(BF16NP)

    # q and k each scaled x4 so fp8e4m3 stays out of subnormals; the
    # softmax scale divides the 16x back out
    Wgs = Wg.copy()
    Wgs[:C, :] *= 4.0
    wg1s = wg1.copy()
    wg1s[:C] *= 4.0
    wA = srwT.reshape(128, 2048)
    qwTa = chunkT(qw_eff)                       # [128, 2, 512]
    qwTa = qwTa.reshape(128, 2, 2, 128)[:, :, :, perm].reshape(128, 512)
    wB1 = np.ascontiguousarray(qwTa)
    wB2 = np.zeros((128, 2432), BF16NP)
    kvwTa = chunkT(Wgs).reshape(128, 2, 4, 128)
    kvwTa[:, :, 0:2, :] = kvwTa[:, :, 0:2, perm]
    wB2[:, 0:1024] = kvwTa.reshape(128, 1024)
    wB2[:, 1024:1536] = chunkT(proj_w).reshape(128, 512)
    wg1a = wg1s.reshape(4, 128).copy()
    wg1a[0:2, :] = wg1a[0:2, perm]
    wB2[0:1, 1536:2048] = wg1a.astype(BF16NP).reshape(1, 512)
    wB2[0, 2048:2176] = BF16NP(1.0)   # ones row: proj-bias + 1/D broadcast
    wB2[0, 2176:2432] = pb_eff.astype(BF16NP)
    wC = np.zeros((128, 4), f32)
    wC[:, 0:2] = pcols(q_b)[perm] * 4.0
    wC[:, 2:4] = pcols(sr_b)
    return dict(wA=np.ascontiguousarray(wA), wB1=wB1, wB2=wB2, wC=wC)


def kernel(x, q_w, q_b, kv_w, kv_b, proj_w, proj_b, a_q, b_q, a_v, b_v,
           sr_w, sr_b, ln_g, ln_b, H, W):
    from concourse.bass_utils import run_bass_kernel_spmd

    x = np.asarray(x, np.float32)
    assert x.shape == (B, N, C) and int(H) == 64 and int(W) == 64

    if "nc" not in _CACHE:
        _CACHE["nc"] = _build_program()
    nc = _CACHE["nc"]

    shared = _prep_shared(q_w, q_b, kv_w, kv_b, proj_w, proj_b, a_q, b_q,
                          a_v, b_v, sr_w, sr_b, ln_g, ln_b)
    in_maps = []
    for c in range(NCORES):
        b, j = c // 4, c % 4
        xb = np.roll(x[b], -NB * j, axis=0)          # own block at rows 0:1024
        xT = np.ascontiguousarray(xb.T.astype(BF16NP))  # [256, 4096]
        xT = np.ascontiguousarray(
            xT.reshape(2, 128, N).transpose(1, 0, 2))   # [128, 2, 4096]
        in_maps.append(dict(shared, xT=xT))

    res = run_bass_kernel_spmd(nc, in_maps, list(range(NCORES)))
    out = np.empty((B, N, C), np.float32)
    for c in range(NCORES):
        b, j = c // 4, c % 4
        out[b, NB * j:NB * (j + 1)] = res.results[c]["out"]
    return out
